# revision 1
# baseline (speedup 1.0000x reference)
"""ALiBi causal attention on 8 TRN2 NeuronCores.

Sharding: core c handles batch b = c//4 and global heads [4*(c%4), 4*(c%4)+4).
Attention is fully local per core; one 8-core AllToAll re-shards the attention
output (head-major -> token-major) for the output projection. Each core emits
512 output rows of its batch; host concatenates.

Host-side input prep: x and the weight slices are pre-transposed (d_model on
the partition axis) and cast to bf16, so the kernel starts matmuls straight
off the DMAs. Wo arrives as a (2048, 1024) "virtual" Wo.T with the other
batch-quad's feature rows zeroed, which makes the post-AllToAll output
projection identical on every core (SPMD) at the cost of a 2x contraction.

Score matmul trick: scores^T[j,i] = (q/8 . k)[j,i] + slope*j - slope*i is one
K=66 float32r matmul: rows 0-63 head dims; kT row 64 = slope*j with ones in
qT row 64; kT row 65 = ones with -slope*i in qT row 65. PSUM gets
scores+bias directly; exp is the only elementwise pass. V carries a ones
column so the PV matmul also emits the softmax denominator (output row 64).
"""

import sys

import numpy as np

try:
    import concourse  # noqa: F401
except ImportError:  # pragma: no cover
    sys.path.insert(0, "/opt/trn_rl_repo")

import ml_dtypes
from concourse import bacc, mybir
import concourse.tile as tile
from concourse.bass_utils import run_bass_kernel_spmd

BF16 = mybir.dt.bfloat16
F32 = mybir.dt.float32
F32R = mybir.dt.float32r

B, T, DM, H = 2, 2048, 1024, 16
D = DM // H            # 64 head dim
NCORES = 8
QUAD = 4               # cores per batch
HPC = 4                # heads per core
PB = 128               # partitions
IC = 512               # i-chunk (query cols per score tile)
JT = 128               # j-tile (key rows per score tile)
NTT = T // PB          # 16 token tiles
NDC = DM // PB         # 8 d_model chunks
FPC = HPC * D          # 256 features per core
VF = NCORES * FPC      # 2048 virtual features after 8-core A2A
TOUT = T // QUAD       # 512 output rows per core
NEG = -1.0e9

import os as _os
PSA = int(_os.environ.get("PSA", 2))
PSS = int(_os.environ.get("PSS", 4))
PSV = int(_os.environ.get("PSV", 2))
EPB = int(_os.environ.get("EPB", 6))

_cache = {}


def _build(sim=False, phase="full", expop=True, nheads=HPC):
    nc = bacc.Bacc("TRN2", target_bir_lowering=False, debug=False,
                   num_devices=NCORES)

    x_e = nc.dram_tensor("x", [DM, T], BF16, kind="ExternalInput")
    wq_e = nc.dram_tensor("wq", [DM, FPC], BF16, kind="ExternalInput")
    wk_e = nc.dram_tensor("wk", [DM, FPC], BF16, kind="ExternalInput")
    wv_e = nc.dram_tensor("wv", [DM, FPC], BF16, kind="ExternalInput")
    wo_e = nc.dram_tensor("wo", [DM, DM], BF16, kind="ExternalInput")
    mask_e = nc.dram_tensor("mask", [PB, PB], F32, kind="ExternalInput")
    kaug_e = nc.dram_tensor("kaug", [6 * HPC, T], BF16, kind="ExternalInput")
    qaug_e = nc.dram_tensor("qaug", [6 * HPC, T], BF16, kind="ExternalInput")
    ones_e = nc.dram_tensor("onesrow", [1, D], BF16, kind="ExternalInput")
    fsel_e = nc.dram_tensor("fsel", [D, 2], F32, kind="ExternalInput")
    out_e = nc.dram_tensor("out", [TOUT, DM], F32, kind="ExternalOutput")

    with tile.TileContext(nc) as tc:
        with (
            tc.tile_pool(name="xt", bufs=8) as xtp,         # xT chunks
            tc.tile_pool(name="wt", bufs=24) as wtp,        # WqkvT chunks
            tc.tile_pool(name="wo", bufs=16) as wop,        # WoT virtual
            tc.tile_pool(name="qk", bufs=8) as qkp,         # qT/kT (66,T) f32
            tc.tile_pool(name="vp", bufs=64) as vp,         # v tiles (128,65)
            tc.tile_pool(name="small", bufs=2) as smp,      # misc small
            tc.tile_pool(name="ep", bufs=EPB) as ep,        # exp tiles
            tc.tile_pool(name="op", bufs=4) as op,          # outT tiles
            tc.tile_pool(name="gp", bufs=16) as gp,         # gathered halves
            tc.tile_pool(name="fo", bufs=2) as fo,          # final out stage
            tc.tile_pool(name="psA", bufs=PSA, space="PSUM") as psA,  # proj
            tc.tile_pool(name="psS", bufs=PSS, space="PSUM") as psS,  # score
            tc.tile_pool(name="psV", bufs=PSV, space="PSUM") as psV,  # pv
            tc.tile_pool(name="dram", bufs=1, space="DRAM") as dp,
        ):
            # ---- constants ----
            mask = smp.tile([PB, PB], F32, tag="mask")
            nc.sync.dma_start(out=mask[:, :], in_=mask_e[:, :])
            onesr = smp.tile([1, D], BF16, tag="ones")
            nc.sync.dma_start(out=onesr[:, :], in_=ones_e[:, :])
            fsel = smp.tile([D, 2], F32, tag="fsel")
            nc.sync.dma_start(out=fsel[:, :], in_=fsel_e[:, :])

            # ---- xT chunks: (128 d, T) bf16, straight DMA ----
            xT = []
            for dc in range(NDC):
                t_ = xtp.tile([PB, T], BF16, tag="xt", name=f"xT{dc}")
                nc.sync.dma_start(out=t_[:, :],
                                  in_=x_e[dc * PB:(dc + 1) * PB, :])
                xT.append(t_)

            # ---- WqkvT chunks: (128 d, 256 f) bf16 ----
            wT = {}
            for wi, w_e in enumerate((wq_e, wk_e, wv_e)):
                wT[wi] = []
                for dc in range(NDC):
                    t_ = wtp.tile([PB, FPC], BF16, tag="wt",
                                  name=f"wT{wi}_{dc}")
                    nc.sync.dma_start(out=t_[:, :],
                                      in_=w_e[dc * PB:(dc + 1) * PB, :])
                    wT[wi].append(t_)

            # ---- projections ----
            # qTt[l]/kTt[l]: (66, T) f32; rows 0-63 data, 64-65 aug rows.
            qTt = [qkp.tile([70, T], BF16, tag="qk", name=f"qT{l}")
                   for l in range(HPC)]
            kTt = [qkp.tile([70, T], BF16, tag="qk", name=f"kT{l}")
                   for l in range(HPC)]
            for l in range(HPC):
                nc.sync.dma_start(out=kTt[l][64:70, :],
                                  in_=kaug_e[6 * l:6 * l + 6, :])
                nc.sync.dma_start(out=qTt[l][64:70, :],
                                  in_=qaug_e[6 * l:6 * l + 6, :])

            # q, k: out (128 f = 2 heads, 512 t) accumulated over d chunks
            def qk_proj(fb):
                for wi, dest, scl in ((0, qTt, 0.125), (1, kTt, 1.0)):
                    for tch in range(T // IC):
                        pp = psA.tile([PB, IC], F32, tag="pp",
                                      name=f"qk{wi}{fb}{tch}")
                        for dc in range(NDC):
                            nc.tensor.matmul(
                                pp[:, :],
                                wT[wi][dc][:, fb * PB:(fb + 1) * PB],
                                xT[dc][:, tch * IC:(tch + 1) * IC],
                                start=(dc == 0), stop=(dc == NDC - 1))
                        for hh in range(2):  # split head pair
                            l = 2 * fb + hh
                            dst = dest[l][0:64, tch * IC:(tch + 1) * IC]
                            if tch % 2 == 0:
                                nc.scalar.mul(dst, pp[hh * D:(hh + 1) * D, :],
                                              scl)
                            else:
                                nc.vector.tensor_scalar_mul(
                                    dst, pp[hh * D:(hh + 1) * D, :], scl)
            qk_proj(0)

            # v natural: (128 t, 256 f) accumulated over d chunks; split into
            # per-head (128, 65) tiles with a ones column at col 64.
            vt = {}
            for l in range(HPC):
                vt[l] = [vp.tile([PB, D + 1], BF16, tag="vp",
                                 name=f"v{l}_{tt}")
                         for tt in range(NTT)]
            for tt in range(NTT):
                pp = psA.tile([PB, FPC], F32, tag="pp")
                for dc in range(NDC):
                    nc.tensor.matmul(pp[:, :],
                                     xT[dc][:, tt * PB:(tt + 1) * PB],
                                     wT[2][dc][:, :],
                                     start=(dc == 0), stop=(dc == NDC - 1))
                for l in range(HPC):
                    if l % 2 == 0:
                        nc.scalar.copy(vt[l][tt][:, 0:D],
                                       pp[:, l * D:(l + 1) * D])
                    else:
                        nc.vector.tensor_copy(vt[l][tt][:, 0:D],
                                              pp[:, l * D:(l + 1) * D])
                    nc.vector.memset(vt[l][tt][:, D:D + 1], 1.0)

            if phase == "proj":
                fot0 = fo.tile([PB, IC], F32, tag="fo")
                nc.vector.tensor_copy(fot0[0:64, :].bitcast(BF16),
                                      qTt[0][0:64, 0:1024])
                for l in range(HPC):
                    nc.vector.tensor_copy(
                        fot0[64:128, :].bitcast(BF16),
                        kTt[l][0:64, 0:1024])
                    nc.vector.tensor_copy(
                        fot0[0:128, 0:32].bitcast(BF16), vt[l][0][:, 0:64])
                nc.sync.dma_start(out=out_e[0:PB, 0:IC], in_=fot0[:, :])

            # ---- attention (head-pair outer, i-chunk inner) ----
            a2a_in = [dp.tile([NCORES, PB, TOUT], BF16, tag="a2ain",
                              name=f"a2ai{u}") for u in range(2)]
            a2a_out = [dp.tile([NCORES, PB, TOUT], BF16, tag="a2aout",
                               name=f"a2ao{u}") for u in range(2)]

            oTs = {}
            oTs2 = {}
            if phase == "proj":
                qk_proj(1)
            for l in range(nheads if phase != "proj" else 0):
                if l == 2:
                    qk_proj(1)
                for ichk in range(T // IC):
                    i0 = ichk * IC
                    njt = i0 // JT + 4           # j-tiles for this i-chunk
                    pv = psV.tile([D + 1, IC], F32, tag="pv")
                    for jt in range(njt):
                        j0 = jt * JT
                        ist = max(i0, j0)        # trim: only i >= j0
                        noff = ist - i0
                        nn = IC - noff
                        sp = psS.tile([PB, IC], F32, tag="sp")
                        nc.tensor.matmul(
                            sp[:, 0:nn],
                            kTt[l][:, j0:j0 + JT],
                            qTt[l][:, ist:i0 + IC],
                            start=True, stop=True)
                        if j0 >= i0:             # diagonal tile: causal mask
                            nc.vector.tensor_add(sp[:, 0:JT], sp[:, 0:JT],
                                                 mask[:, :])
                        et = ep.tile([PB, IC], BF16, tag="ep")
                        if expop:
                            nc.scalar.activation(
                                et[:, 0:nn], sp[:, 0:nn],
                                mybir.ActivationFunctionType.Exp)
                        else:
                            nc.scalar.copy(et[:, 0:nn], sp[:, 0:nn])
                        nc.tensor.matmul(pv[:, noff:IC],
                                         vt[l][jt][:, :],
                                         et[:, 0:nn],
                                         start=(jt == 0), stop=(jt == njt - 1))
                    # normalize: recip of row 64, gpsimd partition bcast
                    rc = smp.tile([1, IC], F32, tag="rc")
                    nc.vector.reciprocal(rc[:, :], pv[D:D + 1, :])
                    bcs = smp.tile([D, IC], F32, tag="bcs")
                    nc.gpsimd.partition_broadcast(bcs[:, :], rc[:, :])
                    u, r = l // 2, (l % 2) * D
                    if r == 0:
                        oTs[ichk] = op.tile([PB, IC], BF16, tag="opa",
                                            name=f"oTa{ichk}_{u}")
                        oTs2[ichk] = op.tile([PB, IC], BF16, tag="opb",
                                             name=f"oTb{ichk}_{u}")
                    # payload x own-quad flag to each slot pair: receivers
                    # sum chunk pairs, so quad selection happens in the data
                    # (fsel per-core constant), keeping the program SPMD
                    nc.vector.scalar_tensor_tensor(
                        oTs[ichk][r:r + D, :], pv[0:D, :], fsel[:, 0:1],
                        bcs[:, :], mybir.AluOpType.mult,
                        mybir.AluOpType.mult)
                    nc.vector.scalar_tensor_tensor(
                        oTs2[ichk][r:r + D, :], pv[0:D, :], fsel[:, 1:2],
                        bcs[:, :], mybir.AluOpType.mult,
                        mybir.AluOpType.mult)
                    if r != 0 or nheads == 1:
                        nc.sync.dma_start(
                            out=a2a_in[u][ichk:ichk + 1, :, :],
                            in_=oTs[ichk][:, :])
                        nc.sync.dma_start(
                            out=a2a_in[u][ichk + 4:ichk + 5, :, :],
                            in_=oTs2[ichk][:, :])
                if l == nheads - 1 and l < 2 and phase != "proj":
                    qk_proj(1)
                if (l % 2 == 1 or nheads == 1) and phase == "full":
                    u = l // 2
                    if sim:
                        nc.gpsimd.dma_start(out=a2a_out[u][:, :, :],
                                            in_=a2a_in[u][:, :, :])
                    else:
                        nc.gpsimd.collective_compute(
                            "AllToAll", mybir.AluOpType.bypass,
                            replica_groups=[list(range(NCORES))],
                            ins=[a2a_in[u].opt()],
                            outs=[a2a_out[u].opt()])
            if phase == "attn":
                nc.gpsimd.dma_start(out=a2a_out[0][0:1, :, :],
                                    in_=a2a_in[0][0:1, :, :])

            # ---- WoT virtual chunks (other-quad rows are zero) ----
            if phase != "full":
                fob = fo.tile([PB, IC], F32, tag="fo")
                if phase == "attn":
                    nc.sync.dma_start(out=fob[:, 0:256].bitcast(BF16),
                                      in_=a2a_out[0][0:1, :, :])
                else:
                    nc.vector.memset(fob[:, :], 0.0)
                nc.sync.dma_start(out=out_e[PB:2 * PB, 0:IC], in_=fob[:, :])
            woT = []
            for fc in range(DM // PB if phase == "full" else 0):
                t_ = wop.tile([PB, DM], BF16, tag="wo", name=f"woT{fc}")
                nc.sync.dma_start(out=t_[:, :],
                                  in_=wo_e[fc * PB:(fc + 1) * PB, :])
                woT.append(t_)

            # ---- gather + output projection (two t-halves) ----
            HT = TOUT // 2                       # 256
            for th in range(2 if phase == "full" else 0):
                gt = []
                for fc in range(DM // PB):       # 8 dense f-chunks
                    ga = gp.tile([PB, HT], BF16, tag="gp", name=f"ga{th}_{fc}")
                    gb = gp.tile([PB, HT], BF16, tag="gp", name=f"gb{th}_{fc}")
                    p, half = fc // 2, fc % 2
                    nc.sync.dma_start(
                        out=ga[:, :],
                        in_=a2a_out[half][p:p + 1, :,
                                          th * HT:(th + 1) * HT])
                    nc.sync.dma_start(
                        out=gb[:, :],
                        in_=a2a_out[half][p + 4:p + 5, :,
                                          th * HT:(th + 1) * HT])
                    gs = gp.tile([PB, HT], BF16, tag="gs", name=f"gs{th}_{fc}")
                    nc.vector.tensor_add(gs[:, :], ga[:, :], gb[:, :])
                    gt.append(gs)
                for ttl in range(HT // PB):      # 2 token tiles per half
                    for oc in range(DM // IC):   # 2 output chunks
                        pp = psA.tile([PB, IC], F32, tag="pp")
                        for fc in range(DM // PB):
                            nc.tensor.matmul(
                                pp[:, :],
                                gt[fc][:, ttl * PB:(ttl + 1) * PB],
                                woT[fc][:, oc * IC:(oc + 1) * IC],
                                start=(fc == 0), stop=(fc == DM // PB - 1))
                        fot = fo.tile([PB, IC], F32, tag="fo")
                        if oc == 0:
                            nc.scalar.copy(fot[:, :], pp[:, :])
                        else:
                            nc.vector.tensor_copy(fot[:, :], pp[:, :])
                        trow = (th * 2 + ttl) * PB
                        nc.sync.dma_start(
                            out=out_e[trow:trow + PB,
                                      oc * IC:(oc + 1) * IC],
                            in_=fot[:, :])

    nc.compile()
    return nc


def _build_env():
    import os
    return _build(sim=bool(os.environ.get("NO_COLL")),
                  phase=os.environ.get("PHASE", "full"),
                  expop=not os.environ.get("NO_EXP"),
                  nheads=int(os.environ.get("NHEADS", HPC)))


def _consts(m):
    """Per-core constant tensors; m = core % 4 (quad rank)."""
    bf = ml_dtypes.bfloat16

    def dec3(v):
        hi = v.astype(bf).astype(np.float32)
        mid = (v - hi).astype(bf).astype(np.float32)
        lo = (v - hi - mid).astype(bf).astype(np.float32)
        return hi, mid, lo

    heads = [4 * m + l for l in range(HPC)]
    slopes = [2.0 ** (-8.0 * (g + 1) / H) for g in heads]
    pos = np.arange(T, dtype=np.float32)
    kaug = np.zeros((6 * HPC, T), np.float32)
    qaug = np.zeros((6 * HPC, T), np.float32)
    for l, s in enumerate(slopes):
        kaug[6 * l:6 * l + 3] = dec3(s * pos)    # slope * j, 3-term exact
        kaug[6 * l + 3:6 * l + 6] = 1.0
        qaug[6 * l:6 * l + 3] = 1.0
        qaug[6 * l + 3:6 * l + 6] = dec3(-s * pos)  # -slope * i
    mask = np.where(np.arange(PB)[None, :] >= np.arange(PB)[:, None],
                    0.0, NEG).astype(np.float32)  # mask[jp, c]: c >= jp valid
    ones = np.ones((1, D), dtype=np.float32)
    fsel = np.zeros((D, 2), np.float32)
    return dict(mask=mask, kaug=kaug.astype(bf), qaug=qaug.astype(bf),
                onesrow=ones.astype(bf), fsel=fsel)


def _in_maps(x, Wq, Wk, Wv, Wo):
    bf = ml_dtypes.bfloat16
    x = np.asarray(x, np.float32)
    xTb = [np.ascontiguousarray(x[b].T).astype(bf) for b in range(B)]
    WqT = np.asarray(Wq, np.float32).T.astype(bf)   # (DM in, DM features)
    WkT = np.asarray(Wk, np.float32).T.astype(bf)
    WvT = np.asarray(Wv, np.float32).T.astype(bf)
    WoT = np.asarray(Wo, np.float32).T.astype(bf)   # (DM f, DM o)
    maps = []
    for c in range(NCORES):
        b, m = c // QUAD, c % QUAD
        fs = FPC * m
        mp = dict(x=xTb[b],
                  wq=np.ascontiguousarray(WqT[:, fs:fs + FPC]),
                  wk=np.ascontiguousarray(WkT[:, fs:fs + FPC]),
                  wv=np.ascontiguousarray(WvT[:, fs:fs + FPC]),
                  wo=WoT, **_consts(m))
        mp["fsel"][:, b] = 1.0
        maps.append(mp)
    return maps


def _assemble(results):
    out = np.zeros((B, T, DM), np.float32)
    for c in range(NCORES):
        b, m = c // QUAD, c % QUAD
        out[b, m * TOUT:(m + 1) * TOUT, :] = results[c]["out"]
    return out


def get_nc():
    if "nc" not in _cache:
        _cache["nc"] = _build()
    return _cache["nc"]


def run(inputs, trace=False, **kw):
    nc = get_nc()
    maps = _in_maps(**inputs)
    res = run_bass_kernel_spmd(nc, maps, core_ids=list(range(NCORES)),
                               trace=trace, **kw)
    return _assemble(res.results), res


def kernel(x, Wq, Wk, Wv, Wo):
    out, _ = run(dict(x=x, Wq=Wq, Wk=Wk, Wv=Wv, Wo=Wo))
    return out



# revision 12
# speedup vs baseline: 1.3167x; 1.3167x over previous
"""ALiBi causal attention on 8 TRN2 NeuronCores.

Sharding: core c handles batch b = c//4 and global heads [4*(c%4), 4*(c%4)+4).
Attention is fully local per core; one 8-core AllToAll re-shards the attention
output (head-major -> token-major) for the output projection. Each core emits
512 output rows of its batch; host concatenates.

Host-side input prep: x and the weight slices are pre-transposed (d_model on
the partition axis) and cast to bf16, so the kernel starts matmuls straight
off the DMAs. Wo arrives as a (2048, 1024) "virtual" Wo.T with the other
batch-quad's feature rows zeroed, which makes the post-AllToAll output
projection identical on every core (SPMD) at the cost of a 2x contraction.

Score matmul trick: scores^T[j,i] = (q/8 . k)[j,i] + slope*j - slope*i is one
K=70 matmul: rows 0-63 head dims; kT rows 64-69 / qT rows 64-69 carry 3-term
bf16 decompositions of slope*j and -slope*i paired with ones. PSUM gets
scores+bias directly; exp is the only elementwise pass. V carries a ones
column so the PV matmul also emits the softmax denominator (output row 64).

Perf structure (vs the first working version):
- j-tiles processed in PAIRS sharing one (128, 1024) 2-bank PSUM score tile,
  halving ScalarE exp instruction count.
- softmax denominators inverted with reciprocal_approx_fast (single DVE op)
  and broadcast across partitions with a K=1 f32r outer-product matmul into
  the spare rows 64:128 of the pv PSUM tile -- gpsimd stays empty so the
  AllToAll triggers fire as soon as their inputs land.
- each AllToAll buffer lives in its own DRAM pool so the u=0 trigger does
  not conservatively wait on u=1's staging writes.
- the output projection is split into half-contractions: the fc-even half
  (fed by AllToAll 0) runs while AllToAll 1 is still in flight; the fc-odd
  half + combine run after.
"""

import sys

import numpy as np

try:
    import concourse  # noqa: F401
except ImportError:  # pragma: no cover
    sys.path.insert(0, "/opt/trn_rl_repo")

import ml_dtypes
from concourse import bacc, mybir
import concourse.tile as tile
from concourse.bass_utils import run_bass_kernel_spmd

BF16 = mybir.dt.bfloat16
F32 = mybir.dt.float32
F32R = mybir.dt.float32r

B, T, DM, H = 2, 2048, 1024, 16
D = DM // H            # 64 head dim
NCORES = 8
QUAD = 4               # cores per batch
HPC = 4                # heads per core
PB = 128               # partitions
IC = 512               # i-chunk (query cols per score tile)
JT = 128               # j-tile (key rows per score tile)
NTT = T // PB          # 16 token tiles
NDC = DM // PB         # 8 d_model chunks
FPC = HPC * D          # 256 features per core
TOUT = T // QUAD       # 512 output rows per core
NEG = -1.0e9

import os as _os
PSA = int(_os.environ.get("PSA", 2))
PSS = int(_os.environ.get("PSS", 2))   # score tiles are 2 banks each now
PSV = int(_os.environ.get("PSV", 2))
EPB = int(_os.environ.get("EPB", 4))

_cache = {}


def _build(sim=False, phase="full", expop=True, nheads=HPC):
    nc = bacc.Bacc("TRN2", target_bir_lowering=False, debug=False,
                   num_devices=NCORES)

    x_e = nc.dram_tensor("x", [DM, T], BF16, kind="ExternalInput")
    wq_e = nc.dram_tensor("wq", [DM, FPC], BF16, kind="ExternalInput")
    wk_e = nc.dram_tensor("wk", [DM, FPC], BF16, kind="ExternalInput")
    wv_e = nc.dram_tensor("wv", [DM, FPC], BF16, kind="ExternalInput")
    wo_e = nc.dram_tensor("wo", [DM, DM], BF16, kind="ExternalInput")
    mask_e = nc.dram_tensor("mask", [PB, PB], F32, kind="ExternalInput")
    kaug_e = nc.dram_tensor("kaug", [6 * HPC, T], BF16, kind="ExternalInput")
    qaug_e = nc.dram_tensor("qaug", [6 * HPC, T], BF16, kind="ExternalInput")
    fsel_e = nc.dram_tensor("fsel", [D, 2], F32, kind="ExternalInput")
    out_e = nc.dram_tensor("out", [TOUT, DM], F32, kind="ExternalOutput")

    from contextlib import ExitStack
    with tile.TileContext(nc) as tc, ExitStack() as es:
            def pool(**kw):
                return es.enter_context(tc.tile_pool(**kw))
            xtp = pool(name="xt", bufs=8)          # xT chunks
            wtp = pool(name="wt", bufs=24)         # WqkvT chunks
            wop = pool(name="wo", bufs=16)         # WoT virtual
            qkp = pool(name="qk", bufs=8)          # qT/kT (70,T)
            vp = pool(name="vp", bufs=64)          # v tiles (128,65)
            smp = pool(name="small", bufs=2)       # misc small
            rcp = pool(name="rcp", bufs=2)         # recip rows
            bcp = pool(name="bcp", bufs=2)         # broadcast recip
            ep = pool(name="ep", bufs=EPB)         # exp tiles
            op = pool(name="op", bufs=4)           # outT tiles
            gp = pool(name="gp", bufs=16)          # gathered halves
            pop = pool(name="po", bufs=8)          # partial0 out
            fo = pool(name="fo", bufs=2)           # final out stage
            psA = pool(name="psA", bufs=PSA, space="PSUM")  # proj
            psS = pool(name="psS", bufs=PSS, space="PSUM")  # score
            psV = pool(name="psV", bufs=PSV, space="PSUM")  # pv
            dpi0 = pool(name="dpi0", bufs=1, space="DRAM")
            dpi1 = pool(name="dpi1", bufs=1, space="DRAM")
            dpo0 = pool(name="dpo0", bufs=1, space="DRAM")
            dpo1 = pool(name="dpo1", bufs=1, space="DRAM")

            # ---- constants ----
            mask = smp.tile([PB, PB], F32, tag="mask")
            nc.sync.dma_start(out=mask[:, :], in_=mask_e[:, :])
            fsel = smp.tile([D, 2], F32, tag="fsel")
            nc.sync.dma_start(out=fsel[:, :], in_=fsel_e[:, :])


            # ---- xT chunks: (128 d, T) bf16, straight DMA ----
            xT = []
            for dc in range(NDC):
                t_ = xtp.tile([PB, T], BF16, tag="xt", name=f"xT{dc}")
                nc.sync.dma_start(out=t_[:, :],
                                  in_=x_e[dc * PB:(dc + 1) * PB, :])
                xT.append(t_)

            # ---- WqkvT chunks: (128 d, 256 f) bf16 ----
            wT = {}
            for wi, w_e in enumerate((wq_e, wk_e, wv_e)):
                wT[wi] = []
                for dc in range(NDC):
                    t_ = wtp.tile([PB, FPC], BF16, tag="wt",
                                  name=f"wT{wi}_{dc}")
                    nc.sync.dma_start(out=t_[:, :],
                                      in_=w_e[dc * PB:(dc + 1) * PB, :])
                    wT[wi].append(t_)

            # ---- projections ----
            # qTt[l]/kTt[l]: (70, T); rows 0-63 data, 64-69 aug rows.
            qTt = [qkp.tile([70, T], BF16, tag="qk", name=f"qT{l}")
                   for l in range(HPC)]
            kTt = [qkp.tile([70, T], BF16, tag="qk", name=f"kT{l}")
                   for l in range(HPC)]
            for l in range(HPC):
                nc.sync.dma_start(out=kTt[l][64:70, :],
                                  in_=kaug_e[6 * l:6 * l + 6, :])
                nc.sync.dma_start(out=qTt[l][64:70, :],
                                  in_=qaug_e[6 * l:6 * l + 6, :])

            # q, k: out (128 f = 2 heads, 512 t) accumulated over d chunks
            def qk_proj(fb):
                for wi, dest, scl in ((0, qTt, 0.125), (1, kTt, 1.0)):
                    for tch in range(T // IC):
                        pp = psA.tile([PB, IC], F32, tag="pp",
                                      name=f"qk{wi}{fb}{tch}")
                        for dc in range(NDC):
                            nc.tensor.matmul(
                                pp[:, :],
                                wT[wi][dc][:, fb * PB:(fb + 1) * PB],
                                xT[dc][:, tch * IC:(tch + 1) * IC],
                                start=(dc == 0), stop=(dc == NDC - 1))
                        for hh in range(2):  # split head pair
                            l = 2 * fb + hh
                            dst = dest[l][0:64, tch * IC:(tch + 1) * IC]
                            if tch % 2 == 0:
                                nc.scalar.mul(dst, pp[hh * D:(hh + 1) * D, :],
                                              scl)
                            else:
                                nc.vector.tensor_scalar_mul(
                                    dst, pp[hh * D:(hh + 1) * D, :], scl)
            qk_proj(0)

            # v natural: (128 t, 256 f) accumulated over d chunks; split into
            # per-head (128, 65) tiles with a ones column at col 64.
            vt = {}
            for l in range(HPC):
                vt[l] = [vp.tile([PB, D + 1], BF16, tag="vp",
                                 name=f"v{l}_{tt}")
                         for tt in range(NTT)]
            for tt in range(NTT):
                pp = psA.tile([PB, FPC], F32, tag="pp")
                for dc in range(NDC):
                    nc.tensor.matmul(pp[:, :],
                                     xT[dc][:, tt * PB:(tt + 1) * PB],
                                     wT[2][dc][:, :],
                                     start=(dc == 0), stop=(dc == NDC - 1))
                for l in range(HPC):
                    if l % 2 == 0:
                        nc.scalar.copy(vt[l][tt][:, 0:D],
                                       pp[:, l * D:(l + 1) * D])
                    else:
                        nc.vector.tensor_copy(vt[l][tt][:, 0:D],
                                              pp[:, l * D:(l + 1) * D])
                    nc.vector.memset(vt[l][tt][:, D:D + 1], 1.0)

            if phase == "proj":
                fot0 = fo.tile([PB, IC], F32, tag="fo")
                nc.vector.tensor_copy(fot0[0:64, :].bitcast(BF16),
                                      qTt[0][0:64, 0:1024])
                for l in range(HPC):
                    nc.vector.tensor_copy(
                        fot0[64:128, :].bitcast(BF16),
                        kTt[l][0:64, 0:1024])
                    nc.vector.tensor_copy(
                        fot0[0:128, 0:32].bitcast(BF16), vt[l][0][:, 0:64])
                nc.sync.dma_start(out=out_e[0:PB, 0:IC], in_=fot0[:, :])

            # ---- attention (head-pair outer, i-chunk inner) ----
            a2a_in = [dpi0.tile([NCORES, PB, TOUT], BF16, tag="a2ain0",
                                name="a2ai0"),
                      dpi1.tile([NCORES, PB, TOUT], BF16, tag="a2ain1",
                                name="a2ai1")]
            a2a_out = [dpo0.tile([NCORES, PB, TOUT], BF16, tag="a2aout0",
                                 name="a2ao0"),
                       dpo1.tile([NCORES, PB, TOUT], BF16, tag="a2aout1",
                                 name="a2ao1")]

            oTs = {}
            oTs2 = {}
            if phase == "proj":
                qk_proj(1)
            for l in range(nheads if phase != "proj" else 0):
                if l == 2:
                    qk_proj(1)
                for ichk in range(T // IC):
                    i0 = ichk * IC
                    njt = i0 // JT + 4           # j-tiles for this i-chunk
                    pv = psV.tile([D + 1, IC], F32, tag="pv")
                    # j-tiles in pairs sharing one 2-bank score tile
                    for jp in range(0, njt, 2):
                        jts = list(range(jp, min(jp + 2, njt)))
                        spp = psS.tile([PB, 2 * IC], F32, tag="sp")
                        et = ep.tile([PB, 2 * IC], BF16, tag="ep")
                        nns = []
                        for h, jt in enumerate(jts):
                            j0 = jt * JT
                            ist = max(i0, j0)    # trim: only i >= j0
                            nn = IC - (ist - i0)
                            nns.append(nn)
                            nc.tensor.matmul(
                                spp[:, h * IC:h * IC + nn],
                                kTt[l][:, j0:j0 + JT],
                                qTt[l][:, ist:i0 + IC],
                                start=True, stop=True)
                            if j0 >= i0:         # diagonal tile: causal mask
                                nc.vector.tensor_add(
                                    spp[:, h * IC:h * IC + JT],
                                    spp[:, h * IC:h * IC + JT], mask[:, :])
                        # exp over contiguous valid spans (no stale reads)
                        if expop:
                            if len(jts) == 2 and nns[0] == IC:
                                nc.scalar.activation(
                                    et[:, 0:IC + nns[1]],
                                    spp[:, 0:IC + nns[1]],
                                    mybir.ActivationFunctionType.Exp)
                            else:
                                for h, jt in enumerate(jts):
                                    nc.scalar.activation(
                                        et[:, h * IC:h * IC + nns[h]],
                                        spp[:, h * IC:h * IC + nns[h]],
                                        mybir.ActivationFunctionType.Exp)
                        else:
                            for h, jt in enumerate(jts):
                                nc.scalar.copy(et[:, h * IC:h * IC + nns[h]],
                                               spp[:, h * IC:h * IC + nns[h]])
                        for h, jt in enumerate(jts):
                            noff = IC - nns[h]
                            nc.tensor.matmul(
                                pv[0:D + 1, noff:IC],
                                vt[l][jt][:, :],
                                et[:, h * IC:h * IC + nns[h]],
                                start=(jt == 0), stop=(jt == njt - 1))
                    # normalize: copy denom row off partition 64, fast recip
                    # (custom-DVE op needs partition-0 operands), then DMA
                    # partition-broadcast into SBUF -- gpsimd stays free so
                    # the collective triggers fire as soon as inputs land
                    dn = rcp.tile([1, IC], F32, tag="dn")
                    nc.vector.tensor_copy(dn[:, :], pv[D:D + 1, :])
                    rc = rcp.tile([1, IC], F32, tag="rc")
                    nc.vector.reciprocal_approx_fast(out=rc[:, :],
                                                     in_=dn[:, :])
                    bcs = bcp.tile([D, IC], F32, tag="bcs")
                    nc.sync.dma_start(
                        out=bcs[:, :],
                        in_=rc[0:1, None, :].broadcast_to([1, D, IC]))
                    u, r = l // 2, (l % 2) * D
                    if r == 0:
                        oTs[ichk] = op.tile([PB, IC], BF16, tag="opa",
                                            name=f"oTa{ichk}_{u}")
                        oTs2[ichk] = op.tile([PB, IC], BF16, tag="opb",
                                             name=f"oTb{ichk}_{u}")
                    # payload x own-quad flag to each slot pair: receivers
                    # sum chunk pairs, so quad selection happens in the data
                    # (fsel per-core constant), keeping the program SPMD
                    nc.vector.scalar_tensor_tensor(
                        oTs[ichk][r:r + D, :], pv[0:D, :], fsel[:, 0:1],
                        bcs[:, :], mybir.AluOpType.mult,
                        mybir.AluOpType.mult)
                    nc.vector.scalar_tensor_tensor(
                        oTs2[ichk][r:r + D, :], pv[0:D, :], fsel[:, 1:2],
                        bcs[:, :], mybir.AluOpType.mult,
                        mybir.AluOpType.mult)
                    if r != 0 or nheads == 1:
                        nc.sync.dma_start(
                            out=a2a_in[u][ichk:ichk + 1, :, :],
                            in_=oTs[ichk][:, :])
                        nc.sync.dma_start(
                            out=a2a_in[u][ichk + 4:ichk + 5, :, :],
                            in_=oTs2[ichk][:, :])
                if (l % 2 == 1 or nheads == 1) and phase == "full":
                    u = l // 2
                    if sim:
                        nc.gpsimd.dma_start(out=a2a_out[u][:, :, :],
                                            in_=a2a_in[u][:, :, :])
                    else:
                        nc.gpsimd.collective_compute(
                            "AllToAll", mybir.AluOpType.bypass,
                            replica_groups=[list(range(NCORES))],
                            ins=[a2a_in[u].opt()],
                            outs=[a2a_out[u].opt()])
            if phase == "attn":
                nc.gpsimd.dma_start(out=a2a_out[0][0:1, :, :],
                                    in_=a2a_in[0][0:1, :, :])

            # ---- WoT virtual chunks (other-quad rows are zero) ----
            if phase != "full":
                fob = fo.tile([PB, IC], F32, tag="fo")
                if phase == "attn":
                    nc.sync.dma_start(out=fob[:, 0:256].bitcast(BF16),
                                      in_=a2a_out[0][0:1, :, :])
                else:
                    nc.vector.memset(fob[:, :], 0.0)
                nc.sync.dma_start(out=out_e[PB:2 * PB, 0:IC], in_=fob[:, :])
            woT = []
            for fc in range(DM // PB if phase == "full" else 0):
                t_ = wop.tile([PB, DM], BF16, tag="wo", name=f"woT{fc}")
                nc.sync.dma_start(out=t_[:, :],
                                  in_=wo_e[fc * PB:(fc + 1) * PB, :])
                woT.append(t_)

            # ---- gather + output projection, split by AllToAll half ----
            # half = fc % 2 selects which AllToAll round carried the chunk;
            # the fc-even partial only needs a2a_out[0], so it runs while
            # AllToAll 1 is still in flight.
            HT = TOUT // 2                       # 256
            def gather(th, fcs):
                gt = {}
                for fc in fcs:
                    p, half = fc // 2, fc % 2
                    ga = gp.tile([PB, HT], BF16, tag="gp",
                                 name=f"ga{th}_{fc}")
                    gb = gp.tile([PB, HT], BF16, tag="gp",
                                 name=f"gb{th}_{fc}")
                    nc.sync.dma_start(
                        out=ga[:, :],
                        in_=a2a_out[half][p:p + 1, :,
                                          th * HT:(th + 1) * HT])
                    nc.sync.dma_start(
                        out=gb[:, :],
                        in_=a2a_out[half][p + 4:p + 5, :,
                                          th * HT:(th + 1) * HT])
                    gs = gp.tile([PB, HT], BF16, tag="gs",
                                 name=f"gs{th}_{fc}")
                    nc.vector.tensor_add(gs[:, :], ga[:, :], gb[:, :])
                    gt[fc] = gs
                return gt

            if phase == "full":
                fcs0, fcs1 = (0, 2, 4, 6), (1, 3, 5, 7)
                pots = {}
                gt0 = {th: gather(th, fcs0) for th in range(2)}
                for th in range(2):
                    for ttl in range(HT // PB):
                        for oc in range(DM // IC):
                            pp = psA.tile([PB, IC], F32, tag="pp")
                            for k, fc in enumerate(fcs0):
                                nc.tensor.matmul(
                                    pp[:, :],
                                    gt0[th][fc][:, ttl * PB:(ttl + 1) * PB],
                                    woT[fc][:, oc * IC:(oc + 1) * IC],
                                    start=(k == 0), stop=(k == 3))
                            pot = pop.tile([PB, IC], F32, tag="po",
                                           name=f"po{th}{ttl}{oc}")
                            if oc == 0:
                                nc.scalar.copy(pot[:, :], pp[:, :])
                            else:
                                nc.vector.tensor_copy(pot[:, :], pp[:, :])
                            pots[(th, ttl, oc)] = pot
                gt1 = {th: gather(th, fcs1) for th in range(2)}
                for th in range(2):
                    for ttl in range(HT // PB):
                        for oc in range(DM // IC):
                            pp = psA.tile([PB, IC], F32, tag="pp")
                            for k, fc in enumerate(fcs1):
                                nc.tensor.matmul(
                                    pp[:, :],
                                    gt1[th][fc][:, ttl * PB:(ttl + 1) * PB],
                                    woT[fc][:, oc * IC:(oc + 1) * IC],
                                    start=(k == 0), stop=(k == 3))
                            fot = fo.tile([PB, IC], F32, tag="fo")
                            nc.vector.tensor_add(fot[:, :], pp[:, :],
                                                 pots[(th, ttl, oc)][:, :])
                            trow = (th * 2 + ttl) * PB
                            nc.sync.dma_start(
                                out=out_e[trow:trow + PB,
                                          oc * IC:(oc + 1) * IC],
                                in_=fot[:, :])

    nc.compile()
    return nc


def _build_env():
    import os
    return _build(sim=bool(os.environ.get("NO_COLL")),
                  phase=os.environ.get("PHASE", "full"),
                  expop=not os.environ.get("NO_EXP"),
                  nheads=int(os.environ.get("NHEADS", HPC)))


def _consts(m):
    """Per-core constant tensors; m = core % 4 (quad rank)."""
    bf = ml_dtypes.bfloat16

    def dec3(v):
        hi = v.astype(bf).astype(np.float32)
        mid = (v - hi).astype(bf).astype(np.float32)
        lo = (v - hi - mid).astype(bf).astype(np.float32)
        return hi, mid, lo

    heads = [4 * m + l for l in range(HPC)]
    slopes = [2.0 ** (-8.0 * (g + 1) / H) for g in heads]
    pos = np.arange(T, dtype=np.float32)
    kaug = np.zeros((6 * HPC, T), np.float32)
    qaug = np.zeros((6 * HPC, T), np.float32)
    for l, s in enumerate(slopes):
        kaug[6 * l:6 * l + 3] = dec3(s * pos)    # slope * j, 3-term exact
        kaug[6 * l + 3:6 * l + 6] = 1.0
        qaug[6 * l:6 * l + 3] = 1.0
        qaug[6 * l + 3:6 * l + 6] = dec3(-s * pos)  # -slope * i
    mask = np.where(np.arange(PB)[None, :] >= np.arange(PB)[:, None],
                    0.0, NEG).astype(np.float32)  # mask[jp, c]: c >= jp valid
    fsel = np.zeros((D, 2), np.float32)
    return dict(mask=mask, kaug=kaug.astype(bf), qaug=qaug.astype(bf),
                fsel=fsel)


def _in_maps(x, Wq, Wk, Wv, Wo):
    bf = ml_dtypes.bfloat16
    x = np.asarray(x, np.float32)
    xTb = [np.ascontiguousarray(x[b].T).astype(bf) for b in range(B)]
    WqT = np.asarray(Wq, np.float32).T.astype(bf)   # (DM in, DM features)
    WkT = np.asarray(Wk, np.float32).T.astype(bf)
    WvT = np.asarray(Wv, np.float32).T.astype(bf)
    WoT = np.asarray(Wo, np.float32).T.astype(bf)   # (DM f, DM o)
    maps = []
    for c in range(NCORES):
        b, m = c // QUAD, c % QUAD
        fs = FPC * m
        mp = dict(x=xTb[b],
                  wq=np.ascontiguousarray(WqT[:, fs:fs + FPC]),
                  wk=np.ascontiguousarray(WkT[:, fs:fs + FPC]),
                  wv=np.ascontiguousarray(WvT[:, fs:fs + FPC]),
                  wo=WoT, **_consts(m))
        mp["fsel"][:, b] = 1.0
        maps.append(mp)
    return maps


def _assemble(results):
    out = np.zeros((B, T, DM), np.float32)
    for c in range(NCORES):
        b, m = c // QUAD, c % QUAD
        out[b, m * TOUT:(m + 1) * TOUT, :] = results[c]["out"]
    return out


def get_nc():
    if "nc" not in _cache:
        _cache["nc"] = _build()
    return _cache["nc"]


def run(inputs, trace=False, **kw):
    nc = get_nc()
    maps = _in_maps(**inputs)
    res = run_bass_kernel_spmd(nc, maps, core_ids=list(range(NCORES)),
                               trace=trace, **kw)
    return _assemble(res.results), res


def kernel(x, Wq, Wk, Wv, Wo):
    out, _ = run(dict(x=x, Wq=Wq, Wk=Wk, Wv=Wv, Wo=Wo))
    return out
